# revision 1
# baseline (speedup 1.0000x reference)
"""Trainium2 Bass kernel: Baichuan attention, tensor-parallel over heads on 8 cores.

Strategy (per core c of 8, handling heads 4c..4c+3):
  Phase 1: QKV projection in transposed layout projT[o, t] = W_c @ x^T using
           fp32r (TF32) matmuls, K-contiguous psum accumulation; results
           staged to DRAM per (batch, o-tile) for fine-grained phase overlap.
  Phase 2: attention per batch: S^T[k,q] = K^T-stationary x Q^T-moving; mask
           blocks classified host-side (skip / free / add); exp on ACT into
           fp32r P^T tiles; PV and row-sum (ones-matmul) accumulate in psum;
           normalize with DVE reciprocal+mul; A^T staged to DRAM.
  Phase 3: partial o_proj out[t, o] = A_c^T.T @ WoT_c; host sums partials.

All matmuls fp32r (full PE rate at 512-wide moving operands, ~TF32 accuracy).
DMAs are batched into large transfers and split across both HWDGE rings
(nc.sync for loads, nc.scalar for stores).
"""
import numpy as np
from contextlib import ExitStack

import concourse.bass as bass
import concourse.tile as tile
from concourse import bacc, mybir
from concourse.bass_utils import run_bass_kernel_spmd

F32R = mybir.dt.float32r
F32 = mybir.dt.float32
EXP = mybir.ActivationFunctionType.Exp

B, S, H = 2, 2048, 4096
NH, HD = 32, 128
T = B * S
NCORES = 8
HPC = NH // NCORES          # heads per core
CCH = HPC * HD              # channels per core (512)
NEG_THRESH = -1e30
SKIP, FREE, ADD = 0, 1, 2

N_TP = 4                    # t-panels of 1024 tokens in phase 1
TPW = T // N_TP             # 1024
N_OT = 12                   # o-tiles of 128 (Q:0-3, K:4-7, V:8-11)
N_HC = H // 128             # 32 h-chunks
N_QC = S // 512             # 4 q-chunks per batch
N_KT = S // 128             # 16 k-tiles per batch


def _build(block_class, rep1=1, rep2=1, rep3=1):
    """block_class[b][qc][kt] in {SKIP, FREE, ADD}."""
    nc = bacc.Bacc("TRN2", target_bir_lowering=False, debug=False,
                   num_devices=NCORES)
    xT = nc.dram_tensor("xT", [H, T], F32R, kind="ExternalInput").ap()
    wT = nc.dram_tensor("wT", [N_OT, N_HC, 128, 128], F32R,
                        kind="ExternalInput").ap()
    maskT = nc.dram_tensor("maskT", [B, S, S], F32, kind="ExternalInput").ap()
    woT = nc.dram_tensor("woT", [CCH, H], F32R, kind="ExternalInput").ap()
    ones_in = nc.dram_tensor("ones", [128, 128], F32R, kind="ExternalInput").ap()
    ident_in = nc.dram_tensor("ident", [128, 128], F32, kind="ExternalInput").ap()
    out = nc.dram_tensor("out", [T, H], F32, kind="ExternalOutput").ap()

    first_kt = [[None] * N_QC for _ in range(B)]
    last_kt = [[None] * N_QC for _ in range(B)]
    for b in range(B):
        for qc in range(N_QC):
            live = [kt for kt in range(N_KT) if block_class[b][qc][kt] != SKIP]
            if live:
                first_kt[b][qc] = live[0]
                last_kt[b][qc] = live[-1]

    max_add = max((sum(1 for kt in range(N_KT) if block_class[b][qc][kt] == ADD)
                   for b in range(B) for qc in range(N_QC)), default=0)
    mask_bufs = max(2, max_add + 2)
    wo_preload = mask_bufs <= 8

    with tile.TileContext(nc) as tc, ExitStack() as top:
        dram = top.enter_context(tc.tile_pool(name="dram", bufs=1, space="DRAM"))
        # per-(b, o-tile) staging for fine-grained cross-phase deps
        proj_stage = [[dram.tile([128, S], F32R if ot < 8 else F32,
                                 tag=f"pst{b}_{ot}", name=f"pst{b}_{ot}")
                       for ot in range(N_OT)] for b in range(B)]
        at_stage = [dram.tile([CCH, S], F32R, tag=f"atst{b}", name=f"atst{b}")
                    for b in range(B)]

        singles = top.enter_context(tc.tile_pool(name="singles", bufs=1))
        ones_sb = singles.tile([128, 128], F32R)
        nc.sync.dma_start(out=ones_sb[:], in_=ones_in)
        ident_sb = singles.tile([128, 128], F32)
        nc.sync.dma_start(out=ident_sb[:], in_=ident_in)

        # ---------------- Phase 1: QKV projection (transposed layout) -------
        for r1 in range(rep1):
            with ExitStack() as ctx:
                xp_pool = ctx.enter_context(
                    tc.tile_pool(name=f"xpanel{r1}", bufs=40))
                w_pool = ctx.enter_context(
                    tc.tile_pool(name=f"wtiles{r1}", bufs=4))
                st_pool = ctx.enter_context(
                    tc.tile_pool(name=f"p1stage{r1}", bufs=6))
                ps_pool = ctx.enter_context(
                    tc.tile_pool(name=f"p1psum{r1}", bufs=6, space="PSUM"))

                for tp in range(N_TP):
                    b = tp // 2
                    tloc = (tp % 2) * TPW
                    t0 = tp * TPW
                    xp = []
                    for hc in range(N_HC):
                        xt = xp_pool.tile([128, TPW], F32R, tag="xp", name="xp")
                        nc.sync.dma_start(
                            out=xt[:],
                            in_=xT[hc * 128:(hc + 1) * 128, t0:t0 + TPW])
                        xp.append(xt)
                    for ot in range(N_OT):
                        whalves = []
                        for wh in range(2):
                            wt = w_pool.tile([128, N_HC // 2, 128], F32R,
                                             tag="wt", name="wt")
                            nc.scalar.dma_start(
                                out=wt[:],
                                in_=wT[ot, wh * 16:(wh + 1) * 16]
                                .transpose([1, 0, 2]))
                            whalves.append(wt)
                        sdt = F32R if ot < 8 else F32
                        pss = [ps_pool.tile([128, 512], F32, tag="ps",
                                            name="ps") for _ in range(2)]
                        for hc in range(N_HC):
                            wsl = whalves[hc // 16][:, hc % 16, :]
                            for tch in range(2):
                                nc.tensor.matmul(
                                    pss[tch][:], lhsT=wsl,
                                    rhs=xp[hc][:, tch * 512:(tch + 1) * 512],
                                    start=(hc == 0), stop=(hc == N_HC - 1))
                        for tch in range(2):
                            stg = st_pool.tile([128, 512], sdt, tag="stg",
                                               name="stg")
                            nc.vector.tensor_copy(out=stg[:], in_=pss[tch][:])
                            nc.scalar.dma_start(
                                out=proj_stage[b][ot][:, tloc + tch * 512:
                                                      tloc + (tch + 1) * 512],
                                in_=stg[:])

        wo_sb = None
        if wo_preload:
            wo_pool = top.enter_context(tc.tile_pool(name="wo_pre", bufs=1))
            wo_sb = []
            for chc in range(HPC):
                w = wo_pool.tile([128, H], F32R, tag=f"wo{chc}",
                                 name=f"wo{chc}")
                nc.scalar.dma_start(
                    out=w[:], in_=woT[chc * 128:(chc + 1) * 128, :])
                wo_sb.append(w)

        # ---------------- Phase 2: attention --------------------------------
        for r2 in range(rep2):
            with ExitStack() as ctx:
                qkv_pool = ctx.enter_context(
                    tc.tile_pool(name=f"qkv{r2}", bufs=4))
                vt_pool = ctx.enter_context(
                    tc.tile_pool(name=f"vtload{r2}", bufs=3))
                mk_pool = ctx.enter_context(
                    tc.tile_pool(name=f"masks{r2}", bufs=mask_bufs))
                pt_pool = ctx.enter_context(
                    tc.tile_pool(name=f"ptiles{r2}", bufs=6))
                at_pool = ctx.enter_context(
                    tc.tile_pool(name=f"atout{r2}", bufs=2))
                zi_pool = ctx.enter_context(
                    tc.tile_pool(name=f"zinv{r2}", bufs=2))
                s_pool = ctx.enter_context(
                    tc.tile_pool(name=f"spsum{r2}", bufs=4, space="PSUM"))
                o_pool = ctx.enter_context(
                    tc.tile_pool(name=f"opsum{r2}", bufs=2, space="PSUM"))
                z_pool = ctx.enter_context(
                    tc.tile_pool(name=f"zpsum{r2}", bufs=2, space="PSUM"))

                for b in range(B):
                    QT, KT, V = [], [], []
                    for hl in range(HPC):
                        qt = qkv_pool.tile([128, S], F32R, tag="qt", name="qt")
                        nc.sync.dma_start(out=qt[:], in_=proj_stage[b][hl][:])
                        QT.append(qt)
                        kt_ = qkv_pool.tile([128, S], F32R, tag="kt", name="kt")
                        nc.sync.dma_start(out=kt_[:],
                                          in_=proj_stage[b][4 + hl][:])
                        KT.append(kt_)
                        v_nat = qkv_pool.tile([128, N_KT, 128], F32R, tag="v",
                                              name="v")
                        for vh in range(2):
                            vt = vt_pool.tile([128, S // 2], F32, tag="vt",
                                              name="vt")
                            nc.sync.dma_start(
                                out=vt[:],
                                in_=proj_stage[b][8 + hl][:, vh * (S // 2):
                                                          (vh + 1) * (S // 2)])
                            for ktl in range(N_KT // 2):
                                kt = vh * (N_KT // 2) + ktl
                                tps = s_pool.tile([128, 128], F32, tag="sps",
                                                  name="sps")
                                nc.tensor.transpose(
                                    tps[:], vt[:, ktl * 128:(ktl + 1) * 128],
                                    ident_sb[:])
                                nc.vector.tensor_copy(out=v_nat[:, kt, :],
                                                      in_=tps[:])
                        V.append(v_nat)

                    for qc in range(N_QC):
                        cls = block_class[b][qc]
                        fkt, lkt = first_kt[b][qc], last_kt[b][qc]
                        mtiles = {}
                        for kt in range(N_KT):
                            if cls[kt] == ADD:
                                mt = mk_pool.tile([128, 512], F32, tag="mk",
                                                  name="mk")
                                nc.sync.dma_start(
                                    out=mt[:],
                                    in_=maskT[b, kt * 128:(kt + 1) * 128,
                                              qc * 512:(qc + 1) * 512])
                                mtiles[kt] = mt
                        for hl in range(HPC):
                            o_tile = o_pool.tile([128, 512], F32, tag="op",
                                                 name="op")
                            z_tile = z_pool.tile([128, 512], F32, tag="zp",
                                                 name="zp")
                            for kt in range(N_KT):
                                if cls[kt] == SKIP:
                                    continue
                                sps = s_pool.tile([128, 512], F32,
                                                  tag="sps", name="sps")
                                nc.tensor.matmul(
                                    sps[:],
                                    lhsT=KT[hl][:, kt * 128:(kt + 1) * 128],
                                    rhs=QT[hl][:, qc * 512:(qc + 1) * 512],
                                    start=True, stop=True)
                                if cls[kt] == ADD:
                                    nc.vector.tensor_add(
                                        sps[:], sps[:], mtiles[kt][:])
                                pt = pt_pool.tile([128, 512], F32R,
                                                  tag="pt", name="pt")
                                nc.scalar.activation(
                                    out=pt[:], in_=sps[:], func=EXP)
                                nc.tensor.matmul(
                                    o_tile[:], lhsT=V[hl][:, kt, :],
                                    rhs=pt[:],
                                    start=(kt == fkt), stop=(kt == lkt))
                                nc.tensor.matmul(
                                    z_tile[:], lhsT=ones_sb[:],
                                    rhs=pt[:],
                                    start=(kt == fkt), stop=(kt == lkt))
                            at = at_pool.tile([128, 512], F32R, tag="at",
                                              name="at")
                            if fkt is None:
                                nc.vector.memset(at[:], 0.0)
                            else:
                                zi = zi_pool.tile([128, 512], F32,
                                                  tag="zi", name="zi")
                                nc.vector.reciprocal(zi[:], z_tile[:])
                                nc.vector.tensor_mul(
                                    at[:], o_tile[:], zi[:])
                            nc.scalar.dma_start(
                                out=at_stage[b][hl * 128:(hl + 1) * 128,
                                                qc * 512:(qc + 1) * 512],
                                in_=at[:])

        # ---------------- Phase 3: o_proj partial ----------------------------
        for r3 in range(rep3):
            with ExitStack() as ctx:
                a_pool = ctx.enter_context(
                    tc.tile_pool(name=f"apan{r3}", bufs=6))
                ob_pool = ctx.enter_context(
                    tc.tile_pool(name=f"obuf{r3}", bufs=4))
                ps3_pool = ctx.enter_context(
                    tc.tile_pool(name=f"p3psum{r3}", bufs=4, space="PSUM"))

                if wo_sb is None:
                    wo_pool = ctx.enter_context(
                        tc.tile_pool(name=f"wo{r3}", bufs=1))
                    wo_sb = []
                    for chc in range(HPC):
                        w = wo_pool.tile([128, H], F32R, tag=f"wo{chc}",
                                         name=f"wo{chc}")
                        nc.sync.dma_start(
                            out=w[:], in_=woT[chc * 128:(chc + 1) * 128, :])
                        wo_sb.append(w)
                for b in range(B):
                    for tt in range(S // 128):
                        apan = a_pool.tile([128, HPC, 128], F32R, tag="ap",
                                           name="ap")
                        nc.sync.dma_start(
                            out=apan[:],
                            in_=at_stage[b][:, tt * 128:(tt + 1) * 128]
                            .rearrange("(c p) t -> p c t", p=128))
                        t0 = b * S + tt * 128
                        ob = ob_pool.tile([128, H], F32, tag="ob", name="ob")
                        for oc in range(H // 512):
                            ps = ps3_pool.tile([128, 512], F32, tag="ps3",
                                               name="ps3")
                            for chc in range(HPC):
                                nc.tensor.matmul(
                                    ps[:], lhsT=apan[:, chc, :],
                                    rhs=wo_sb[chc][:, oc * 512:(oc + 1) * 512],
                                    start=(chc == 0), stop=(chc == HPC - 1))
                            nc.scalar.copy(ob[:, oc * 512:(oc + 1) * 512],
                                           ps[:])
                        nc.scalar.dma_start(out=out[t0:t0 + 128, :], in_=ob[:])

    nc.compile()
    return nc


def _classify_mask(attention_mask):
    """Per (b, qc, kt) block class for maskT blocks [128 k, 512 q]."""
    m = np.asarray(attention_mask)[:, 0]          # [B, q, k]
    mT = m.transpose(0, 2, 1)                     # [B, k, q]
    blk = mT.reshape(B, N_KT, 128, N_QC, 512)
    mx = blk.max(axis=(2, 4))                     # [B, kt, qc]
    mn = blk.min(axis=(2, 4))
    cls = np.full((B, N_QC, N_KT), ADD, dtype=np.int64)
    free = (mx == 0.0) & (mn == 0.0)
    skip = mx <= NEG_THRESH
    for b in range(B):
        for qc in range(N_QC):
            for kt in range(N_KT):
                if free[b, kt, qc]:
                    cls[b, qc, kt] = FREE
                elif skip[b, kt, qc]:
                    cls[b, qc, kt] = SKIP
    return cls, np.ascontiguousarray(mT.astype(np.float32))


_CACHE = {}


def kernel(hidden_states, attention_mask, W_pack, o_proj_w):
    hidden_states = np.asarray(hidden_states, dtype=np.float32)
    attention_mask = np.asarray(attention_mask, dtype=np.float32)
    W_pack = np.asarray(W_pack, dtype=np.float32)
    o_proj_w = np.asarray(o_proj_w, dtype=np.float32)

    cls, maskT = _classify_mask(attention_mask)
    key = cls.tobytes()
    if key not in _CACHE:
        _CACHE[key] = _build(cls.tolist())
    nc = _CACHE[key]

    x2d = hidden_states.reshape(T, H)
    xT = np.ascontiguousarray(x2d.T)              # [H, T]
    ones = np.ones((128, 128), dtype=np.float32)
    ident = np.eye(128, dtype=np.float32)
    scale = np.float32(1.0 / np.sqrt(HD))

    in_maps = []
    for c in range(NCORES):
        r0 = c * CCH
        wq = W_pack[r0:r0 + CCH, :] * scale       # fold softmax scale into Q
        wk = W_pack[H + r0:H + r0 + CCH, :]
        wv = W_pack[2 * H + r0:2 * H + r0 + CCH, :]
        w_c = np.concatenate([wq, wk, wv], axis=0)       # [1536, H]
        wt_tiled = np.ascontiguousarray(
            w_c.T.reshape(N_HC, 128, N_OT, 128).transpose(2, 0, 1, 3))
        woT_c = np.ascontiguousarray(o_proj_w[:, r0:r0 + CCH].T)  # [CCH, H]
        in_maps.append({
            "xT": xT, "wT": wt_tiled, "maskT": maskT, "woT": woT_c,
            "ones": ones, "ident": ident,
        })

    res = run_bass_kernel_spmd(nc, in_maps, core_ids=list(range(NCORES)))
    acc = res.results[0]["out"].astype(np.float32)
    for c in range(1, NCORES):
        acc = acc + res.results[c]["out"]
    return acc.reshape(B, S, H)



# revision 2
# speedup vs baseline: 23962.6659x; 23962.6659x over previous
"""Trainium2 Bass kernel: Baichuan attention, tensor-parallel over heads on 8 cores.

Strategy (per core c of 8, handling heads 4c..4c+3):
  Phase 1: QKV projection in transposed layout projT[o, t] = W_c @ x^T using
           fp32r (TF32) matmuls, K-contiguous psum accumulation; results
           staged to DRAM per (batch, o-tile) for fine-grained phase overlap.
  Phase 2: attention per batch: S^T[k,q] = K^T-stationary x Q^T-moving; mask
           blocks classified host-side (skip / free / add); exp on ACT into
           fp32r P^T tiles; PV and row-sum (ones-matmul) accumulate in psum;
           normalize with DVE reciprocal+mul; A^T staged to DRAM.
  Phase 3: partial o_proj out[t, o] = A_c^T.T @ WoT_c; host sums partials.

All matmuls fp32r (full PE rate at 512-wide moving operands, ~TF32 accuracy).
DMAs are batched into large transfers and split across both HWDGE rings
(nc.sync for loads, nc.scalar for stores).
"""
import numpy as np
from contextlib import ExitStack

import concourse.bass as bass
import concourse.tile as tile
from concourse import bacc, mybir
from concourse.bass_utils import run_bass_kernel_spmd

F32R = mybir.dt.float32r
F32 = mybir.dt.float32
EXP = mybir.ActivationFunctionType.Exp

B, S, H = 2, 2048, 4096
NH, HD = 32, 128
T = B * S
NCORES = 8
HPC = NH // NCORES          # heads per core
CCH = HPC * HD              # channels per core (512)
NEG_THRESH = -1e30
SKIP, FREE, ADD = 0, 1, 2

N_TP = 4                    # t-panels of 1024 tokens in phase 1
TPW = T // N_TP             # 1024
N_OT = 12                   # o-tiles of 128 (Q:0-3, K:4-7, V:8-11)
N_HC = H // 128             # 32 h-chunks
N_QC = S // 512             # 4 q-chunks per batch
N_KT = S // 128             # 16 k-tiles per batch


def _build(block_class, rep1=1, rep2=1, rep3=1):
    """block_class[b][qc][kt] in {SKIP, FREE, ADD}."""
    nc = bacc.Bacc("TRN2", target_bir_lowering=False, debug=False,
                   num_devices=NCORES)
    xT = nc.dram_tensor("xT", [H, T], F32R, kind="ExternalInput").ap()
    wT = nc.dram_tensor("wT", [N_OT, N_HC, 128, 128], F32R,
                        kind="ExternalInput").ap()
    maskT = nc.dram_tensor("maskT", [B, S, S], F32, kind="ExternalInput").ap()
    woT = nc.dram_tensor("woT", [CCH, H], F32R, kind="ExternalInput").ap()
    ones_in = nc.dram_tensor("ones", [128, 128], F32R, kind="ExternalInput").ap()
    ident_in = nc.dram_tensor("ident", [128, 128], F32, kind="ExternalInput").ap()
    out = nc.dram_tensor("out", [T, H], F32, kind="ExternalOutput").ap()

    first_kt = [[None] * N_QC for _ in range(B)]
    last_kt = [[None] * N_QC for _ in range(B)]
    for b in range(B):
        for qc in range(N_QC):
            live = [kt for kt in range(N_KT) if block_class[b][qc][kt] != SKIP]
            if live:
                first_kt[b][qc] = live[0]
                last_kt[b][qc] = live[-1]

    max_add = max((sum(1 for kt in range(N_KT) if block_class[b][qc][kt] == ADD)
                   for b in range(B) for qc in range(N_QC)), default=0)
    mask_bufs = max(2, max_add + 2)
    wo_preload = mask_bufs <= 8

    with tile.TileContext(nc) as tc, ExitStack() as top:
        dram = top.enter_context(tc.tile_pool(name="dram", bufs=1, space="DRAM"))
        # per-(b, o-tile) staging for fine-grained cross-phase deps
        proj_stage = [[dram.tile([128, S], F32R if ot < 8 else F32,
                                 tag=f"pst{b}_{ot}", name=f"pst{b}_{ot}")
                       for ot in range(N_OT)] for b in range(B)]
        at_stage = [dram.tile([CCH, S], F32R, tag=f"atst{b}", name=f"atst{b}")
                    for b in range(B)]

        singles = top.enter_context(tc.tile_pool(name="singles", bufs=1))
        ones_sb = singles.tile([128, 128], F32R)
        nc.sync.dma_start(out=ones_sb[:], in_=ones_in)
        ident_sb = singles.tile([128, 128], F32)
        nc.sync.dma_start(out=ident_sb[:], in_=ident_in)

        # ---------------- Phase 1: QKV projection (transposed layout) -------
        for r1 in range(rep1):
            with ExitStack() as ctx:
                xp_pool = ctx.enter_context(
                    tc.tile_pool(name=f"xpanel{r1}", bufs=40))
                w_pool = ctx.enter_context(
                    tc.tile_pool(name=f"wtiles{r1}", bufs=4))
                st_pool = ctx.enter_context(
                    tc.tile_pool(name=f"p1stage{r1}", bufs=6))
                ps_pool = ctx.enter_context(
                    tc.tile_pool(name=f"p1psum{r1}", bufs=6, space="PSUM"))

                for tp in range(N_TP):
                    b = tp // 2
                    tloc = (tp % 2) * TPW
                    t0 = tp * TPW
                    xp = []
                    for hc in range(N_HC):
                        xt = xp_pool.tile([128, TPW], F32R, tag="xp", name="xp")
                        nc.sync.dma_start(
                            out=xt[:],
                            in_=xT[hc * 128:(hc + 1) * 128, t0:t0 + TPW])
                        xp.append(xt)
                    for ot in range(N_OT):
                        whalves = []
                        for wh in range(2):
                            wt = w_pool.tile([128, N_HC // 2, 128], F32R,
                                             tag="wt", name="wt")
                            nc.scalar.dma_start(
                                out=wt[:],
                                in_=wT[ot, wh * 16:(wh + 1) * 16]
                                .transpose([1, 0, 2]))
                            whalves.append(wt)
                        sdt = F32R if ot < 8 else F32
                        pss = [ps_pool.tile([128, 512], F32, tag="ps",
                                            name="ps") for _ in range(2)]
                        for hc in range(N_HC):
                            wsl = whalves[hc // 16][:, hc % 16, :]
                            for tch in range(2):
                                nc.tensor.matmul(
                                    pss[tch][:], lhsT=wsl,
                                    rhs=xp[hc][:, tch * 512:(tch + 1) * 512],
                                    start=(hc == 0), stop=(hc == N_HC - 1))
                        for tch in range(2):
                            stg = st_pool.tile([128, 512], sdt, tag="stg",
                                               name="stg")
                            nc.vector.tensor_copy(out=stg[:], in_=pss[tch][:])
                            nc.scalar.dma_start(
                                out=proj_stage[b][ot][:, tloc + tch * 512:
                                                      tloc + (tch + 1) * 512],
                                in_=stg[:])

        wo_sb = None
        if wo_preload:
            wo_pool = top.enter_context(tc.tile_pool(name="wo_pre", bufs=1))
            wo_sb = []
            for chc in range(HPC):
                w = wo_pool.tile([128, H], F32R, tag=f"wo{chc}",
                                 name=f"wo{chc}")
                nc.scalar.dma_start(
                    out=w[:], in_=woT[chc * 128:(chc + 1) * 128, :])
                wo_sb.append(w)

        # ---------------- Phase 2: attention --------------------------------
        for r2 in range(rep2):
            with ExitStack() as ctx:
                qkv_pool = ctx.enter_context(
                    tc.tile_pool(name=f"qkv{r2}", bufs=4))
                vt_pool = ctx.enter_context(
                    tc.tile_pool(name=f"vtload{r2}", bufs=3))
                mk_pool = ctx.enter_context(
                    tc.tile_pool(name=f"masks{r2}", bufs=mask_bufs))
                pt_pool = ctx.enter_context(
                    tc.tile_pool(name=f"ptiles{r2}", bufs=6))
                at_pool = ctx.enter_context(
                    tc.tile_pool(name=f"atout{r2}", bufs=2))
                zi_pool = ctx.enter_context(
                    tc.tile_pool(name=f"zinv{r2}", bufs=2))
                s_pool = ctx.enter_context(
                    tc.tile_pool(name=f"spsum{r2}", bufs=4, space="PSUM"))
                o_pool = ctx.enter_context(
                    tc.tile_pool(name=f"opsum{r2}", bufs=2, space="PSUM"))
                z_pool = ctx.enter_context(
                    tc.tile_pool(name=f"zpsum{r2}", bufs=2, space="PSUM"))

                for b in range(B):
                    QT, KT, V = [], [], []
                    for hl in range(HPC):
                        qt = qkv_pool.tile([128, S], F32R, tag="qt", name="qt")
                        nc.sync.dma_start(out=qt[:], in_=proj_stage[b][hl][:])
                        QT.append(qt)
                        kt_ = qkv_pool.tile([128, S], F32R, tag="kt", name="kt")
                        nc.sync.dma_start(out=kt_[:],
                                          in_=proj_stage[b][4 + hl][:])
                        KT.append(kt_)
                        v_nat = qkv_pool.tile([128, N_KT, 128], F32R, tag="v",
                                              name="v")
                        for vh in range(2):
                            vt = vt_pool.tile([128, S // 2], F32, tag="vt",
                                              name="vt")
                            nc.sync.dma_start(
                                out=vt[:],
                                in_=proj_stage[b][8 + hl][:, vh * (S // 2):
                                                          (vh + 1) * (S // 2)])
                            for ktl in range(N_KT // 2):
                                kt = vh * (N_KT // 2) + ktl
                                tps = s_pool.tile([128, 128], F32, tag="sps",
                                                  name="sps")
                                nc.tensor.transpose(
                                    tps[:], vt[:, ktl * 128:(ktl + 1) * 128],
                                    ident_sb[:])
                                nc.vector.tensor_copy(out=v_nat[:, kt, :],
                                                      in_=tps[:])
                        V.append(v_nat)

                    for qc in range(N_QC):
                        cls = block_class[b][qc]
                        fkt, lkt = first_kt[b][qc], last_kt[b][qc]
                        mtiles = {}
                        for kt in range(N_KT):
                            if cls[kt] == ADD:
                                mt = mk_pool.tile([128, 512], F32, tag="mk",
                                                  name="mk")
                                nc.sync.dma_start(
                                    out=mt[:],
                                    in_=maskT[b, kt * 128:(kt + 1) * 128,
                                              qc * 512:(qc + 1) * 512])
                                mtiles[kt] = mt
                        for hl in range(HPC):
                            o_tile = o_pool.tile([128, 512], F32, tag="op",
                                                 name="op")
                            z_tile = z_pool.tile([128, 512], F32, tag="zp",
                                                 name="zp")
                            for kt in range(N_KT):
                                if cls[kt] == SKIP:
                                    continue
                                sps = s_pool.tile([128, 512], F32,
                                                  tag="sps", name="sps")
                                nc.tensor.matmul(
                                    sps[:],
                                    lhsT=KT[hl][:, kt * 128:(kt + 1) * 128],
                                    rhs=QT[hl][:, qc * 512:(qc + 1) * 512],
                                    start=True, stop=True)
                                if cls[kt] == ADD:
                                    nc.vector.tensor_add(
                                        sps[:], sps[:], mtiles[kt][:])
                                pt = pt_pool.tile([128, 512], F32R,
                                                  tag="pt", name="pt")
                                nc.scalar.activation(
                                    out=pt[:], in_=sps[:], func=EXP)
                                nc.tensor.matmul(
                                    o_tile[:], lhsT=V[hl][:, kt, :],
                                    rhs=pt[:],
                                    start=(kt == fkt), stop=(kt == lkt))
                                nc.tensor.matmul(
                                    z_tile[:], lhsT=ones_sb[:],
                                    rhs=pt[:],
                                    start=(kt == fkt), stop=(kt == lkt))
                            at = at_pool.tile([128, 512], F32R, tag="at",
                                              name="at")
                            if fkt is None:
                                nc.vector.memset(at[:], 0.0)
                            else:
                                zi = zi_pool.tile([128, 512], F32,
                                                  tag="zi", name="zi")
                                nc.vector.reciprocal(zi[:], z_tile[:])
                                nc.vector.tensor_mul(
                                    at[:], o_tile[:], zi[:])
                            nc.scalar.dma_start(
                                out=at_stage[b][hl * 128:(hl + 1) * 128,
                                                qc * 512:(qc + 1) * 512],
                                in_=at[:])

        # ---------------- Phase 3: o_proj partial ----------------------------
        for r3 in range(rep3):
            with ExitStack() as ctx:
                a_pool = ctx.enter_context(
                    tc.tile_pool(name=f"apan{r3}", bufs=6))
                ob_pool = ctx.enter_context(
                    tc.tile_pool(name=f"obuf{r3}", bufs=4))
                ps3_pool = ctx.enter_context(
                    tc.tile_pool(name=f"p3psum{r3}", bufs=4, space="PSUM"))

                if wo_sb is None:
                    wo_pool = ctx.enter_context(
                        tc.tile_pool(name=f"wo{r3}", bufs=1))
                    wo_sb = []
                    for chc in range(HPC):
                        w = wo_pool.tile([128, H], F32R, tag=f"wo{chc}",
                                         name=f"wo{chc}")
                        nc.sync.dma_start(
                            out=w[:], in_=woT[chc * 128:(chc + 1) * 128, :])
                        wo_sb.append(w)
                for b in range(B):
                    for tt in range(S // 128):
                        apan = a_pool.tile([128, HPC, 128], F32R, tag="ap",
                                           name="ap")
                        nc.sync.dma_start(
                            out=apan[:],
                            in_=at_stage[b][:, tt * 128:(tt + 1) * 128]
                            .rearrange("(c p) t -> p c t", p=128))
                        t0 = b * S + tt * 128
                        ob = ob_pool.tile([128, H], F32, tag="ob", name="ob")
                        for oc in range(H // 512):
                            ps = ps3_pool.tile([128, 512], F32, tag="ps3",
                                               name="ps3")
                            for chc in range(HPC):
                                nc.tensor.matmul(
                                    ps[:], lhsT=apan[:, chc, :],
                                    rhs=wo_sb[chc][:, oc * 512:(oc + 1) * 512],
                                    start=(chc == 0), stop=(chc == HPC - 1))
                            nc.scalar.copy(ob[:, oc * 512:(oc + 1) * 512],
                                           ps[:])
                        nc.scalar.dma_start(out=out[t0:t0 + 128, :], in_=ob[:])

    nc.compile()
    return nc


def _classify_mask(attention_mask):
    """Per (b, qc, kt) block class for maskT blocks [128 k, 512 q]."""
    m = np.asarray(attention_mask)[:, 0]          # [B, q, k]
    mT = m.transpose(0, 2, 1)                     # [B, k, q]
    blk = mT.reshape(B, N_KT, 128, N_QC, 512)
    mx = blk.max(axis=(2, 4))                     # [B, kt, qc]
    mn = blk.min(axis=(2, 4))
    cls = np.full((B, N_QC, N_KT), ADD, dtype=np.int64)
    free = (mx == 0.0) & (mn == 0.0)
    skip = mx <= NEG_THRESH
    for b in range(B):
        for qc in range(N_QC):
            for kt in range(N_KT):
                if free[b, kt, qc]:
                    cls[b, qc, kt] = FREE
                elif skip[b, kt, qc]:
                    cls[b, qc, kt] = SKIP
    return cls, np.ascontiguousarray(mT.astype(np.float32))


_CACHE = {}


def _prepare(hidden_states, attention_mask, W_pack, o_proj_w):
    """Build (nc, in_maps); shared by kernel() and the profiling harness."""
    hidden_states = np.asarray(hidden_states, dtype=np.float32)
    attention_mask = np.asarray(attention_mask, dtype=np.float32)
    W_pack = np.asarray(W_pack, dtype=np.float32)
    o_proj_w = np.asarray(o_proj_w, dtype=np.float32)

    cls, maskT = _classify_mask(attention_mask)
    key = cls.tobytes()
    if key not in _CACHE:
        _CACHE[key] = _build(cls.tolist())
    nc = _CACHE[key]

    x2d = hidden_states.reshape(T, H)
    xT = np.ascontiguousarray(x2d.T)              # [H, T]
    ones = np.ones((128, 128), dtype=np.float32)
    ident = np.eye(128, dtype=np.float32)
    scale = np.float32(1.0 / np.sqrt(HD))

    in_maps = []
    for c in range(NCORES):
        r0 = c * CCH
        wq = W_pack[r0:r0 + CCH, :] * scale       # fold softmax scale into Q
        wk = W_pack[H + r0:H + r0 + CCH, :]
        wv = W_pack[2 * H + r0:2 * H + r0 + CCH, :]
        w_c = np.concatenate([wq, wk, wv], axis=0)       # [1536, H]
        wt_tiled = np.ascontiguousarray(
            w_c.T.reshape(N_HC, 128, N_OT, 128).transpose(2, 0, 1, 3))
        woT_c = np.ascontiguousarray(o_proj_w[:, r0:r0 + CCH].T)  # [CCH, H]
        in_maps.append({
            "xT": xT, "wT": wt_tiled, "maskT": maskT, "woT": woT_c,
            "ones": ones, "ident": ident,
        })
    return nc, in_maps


def _finish(res):
    acc = res.results[0]["out"].astype(np.float32)
    for c in range(1, NCORES):
        acc = acc + res.results[c]["out"]
    return acc.reshape(B, S, H)


def kernel(hidden_states, attention_mask, W_pack, o_proj_w):
    nc, in_maps = _prepare(hidden_states, attention_mask, W_pack, o_proj_w)
    res = run_bass_kernel_spmd(nc, in_maps, core_ids=list(range(NCORES)))
    return _finish(res)

